# revision 39
# baseline (speedup 1.0000x reference)
"""BitSelfAttention on 8 TRN2 NeuronCores — fp8 DoubleRow hybrid.

Sharding: core c handles batch b = c//2 and head-group hg = c%2 (8 of 16
heads). Each core computes its 8 heads' QKV projections + causal attention +
its slice of the o_proj GEMM, producing a partial output ([D, T], fp32);
host sums the two head-group partials per batch and applies the folded
BitLinear gammas.

The device works in pure-ternary units: BitLinear weights are sent as their
ternary {-1,0,+1} values (EXACT in fp8e4), gamma_q*gamma_k is folded into the
softmax exp scale, and 2*gamma_v*gamma_o is applied on the host (the V
weights carry an extra 0.5 — exact in bf16 — so that |v~| stays < 240, the
TRN fp8e4 saturation point).

fp8 DoubleRow (2 contraction subtiles per MM, ~1.8x MAC throughput) is used
where a numpy error simulation showed it is safe (sim rel err 0.92% vs the
2e-2 gate; all-fp8 variants fail):
  - Q/K projections:  fp8 DR (x as e4m3 moving operand, ternary w stationary)
  - V projection:     bf16, x STATIONARY per token-tile so V lands
                      token-major directly — kills the 128 PE transposes the
                      baseline spent ~35us on. w_v is the moving operand.
  - QK^T scores:      bf16 (contraction is d_head=128 — DR inapplicable)
  - P@V off-diagonal: fp8 DR (exp emits e4m3, V kept in an fp8 copy)
  - P@V diagonal:     bf16 (peaked early-token rows need accurate V)
  - o_proj:           tokens 0-127 bf16 (the absmax-setting peaked rows),
                      tokens 128+ fp8 DR
Scheduling: TWO heads' attention runs staggered, each block split into
phase A (all S matmuls + exps; P parked in SBUF) and phase B (all PV /
row-sum matmuls) so a backlogged activation engine can never stall the PE;
projection/V/o_proj fill chains are pumped between steps. Off-diagonal
row-sums ride the PE (fp8-DR all-ones stationary into a PSUM bank);
diagonal partials accumulate on DVE and one ones-matmul per block does the
cross-partition reduce+broadcast; normalization via fast reciprocal.
x8 is loaded as one tile per DR pair so the first projection matmul starts
after ~1/8 of the transfer; outputs stream back as bf16.
"""

import math

import ml_dtypes
import numpy as np

import concourse.mybir as mybir
import concourse.tile as tile
from concourse import bacc
from concourse import bass_utils

BF16 = mybir.dt.bfloat16
F32 = mybir.dt.float32
F8 = mybir.dt.float8e4
DR = mybir.MatmulPerfMode.DoubleRow
EXP = mybir.ActivationFunctionType.Exp

D_MODEL = 2048
N_HEAD = 16
D_HEAD = 128
B = 4
T_FULL = 2048
N_CORES = 8
F_LOC = D_MODEL // 2  # features per core (8 heads)


def build_bass(scale, T=T_FULL, D=D_MODEL, F=F_LOC, debug=False):
    """Build the single-core program (SPMD across 8 cores via input data)."""
    P = 128
    KD = D // P      # contraction 128-tiles (16)
    JD = KD // 2     # DR pairs over contraction (8)
    TT = T // P      # token 128-tiles
    TB = T // 512    # token 512-blocks
    H = F // P       # local heads (8)
    MT = D // P      # output-dmodel 128-tiles (16)
    KT_PER_B = 512 // P

    nc = bacc.Bacc("TRN2", target_bir_lowering=False, debug=debug,
                   num_devices=N_CORES)
    # x8/wvm arrive pre-arranged in their exact SBUF layouts (single DMA)
    x8_d = nc.dram_tensor("x8", [P, KD * T], F8, kind="ExternalInput").ap()
    xtc_d = nc.dram_tensor("xtc", [TT, P, KD * P], BF16,
                           kind="ExternalInput").ap()
    # wq8/wk8: [H, 128, KD*128] ternary fp8, [h, p, kd*128+f] = t[h*128+f, kd*128+p]
    wq8_d = nc.dram_tensor("wq8", [H, P, KD * P], F8, kind="ExternalInput").ap()
    wk8_d = nc.dram_tensor("wk8", [H, P, KD * P], F8, kind="ExternalInput").ap()
    # wvm: [128, KD*F] bf16, [p, kd*F+f] = 0.5 * tv[f, kd*128+p] (moving op)
    wvm_d = nc.dram_tensor("wvm", [P, KD * F], BF16, kind="ExternalInput").ap()
    # wo: [MT, 128, H*128], [m, p, h*128+j] = to[m*128+j, h*128+p]
    wo8_d = nc.dram_tensor("wo8", [MT, P, H * P], F8, kind="ExternalInput").ap()
    wob_d = nc.dram_tensor("wob", [MT, P, H * P], BF16,
                           kind="ExternalInput").ap()
    cm_d = nc.dram_tensor("cmask", [P, P], BF16, kind="ExternalInput").ap()
    out_d = nc.dram_tensor("outT", [D, T], BF16, kind="ExternalOutput").ap()

    with tile.TileContext(nc) as tc:
        with (
            tc.tile_pool(name="big", bufs=1) as big,
            tc.tile_pool(name="work", bufs=2) as work,
            tc.tile_pool(name="psS", bufs=2, space="PSUM") as psS,
            tc.tile_pool(name="psO", bufs=2, space="PSUM") as psO,
            tc.tile_pool(name="psR", bufs=2, space="PSUM") as psR,
            tc.tile_pool(name="psP", bufs=2, space="PSUM") as psP,
        ):
            # ---- persistent inputs (head-0 weights first: first MMs need them)
            def load_qk_weights(h):
                wq_t = work.tile([P, KD, P], F8, name=f"wq{h}", tag="wq8",
                                 bufs=3)
                nc.sync.dma_start(out=wq_t.rearrange("p kd f -> p (kd f)"),
                                  in_=wq8_d[h])
                wk_t = work.tile([P, KD, P], F8, name=f"wk{h}", tag="wk8",
                                 bufs=3)
                nc.sync.dma_start(out=wk_t.rearrange("p kd f -> p (kd f)"),
                                  in_=wk8_d[h])
                return wq_t, wk_t

            ws_list = [None] * (H + 3)
            ws_list[0] = load_qk_weights(0)
            # x8 as one tile per DR kd-pair: dependencies stay per-pair, so
            # the first projection matmul starts after ~1/8 of the transfer
            x8p = [big.tile([P, 2, T], F8, name=f"x8p{j}", tag="x8p", bufs=JD)
                   for j in range(JD)]
            for j in range(JD):
                nc.sync.dma_start(
                    out=x8p[j].rearrange("p a t -> p (a t)"),
                    in_=x8_d[:, 2 * j * T:(2 * j + 2) * T])
            if H > 1:
                ws_list[1] = load_qk_weights(1)
            if H > 2:
                ws_list[2] = load_qk_weights(2)
            wv_sb = big.tile([P, KD, F], BF16, name="wv_sb", tag="wv", bufs=1)
            nc.sync.dma_start(
                out=wv_sb.rearrange("p kd f -> p (kd f)"), in_=wvm_d)
            ones = big.tile([P, P], BF16, name="ones_sb", tag="ones", bufs=1)
            nc.vector.memset(ones, 1.0)
            ones8 = big.tile([P, 2, P], F8, name="ones8_sb", tag="ones8",
                             bufs=1)
            nc.vector.memset(ones8, 1.0)
            cmask = big.tile([P, P], BF16, name="cmask_sb", tag="cmask", bufs=1)
            nc.sync.dma_start(out=cmask, in_=cm_d)
            vhb = big.tile([P, TT, F], BF16, name="vhb", tag="vhb", bufs=1)
            vh8 = big.tile([P, TT, F], F8, name="vh8", tag="vh8", bufs=1)
            # only tokens 0-127 (the sharply peaked rows) keep a bf16 o_proj
            # path; everything else consumes the fp8 copy
            otb = big.tile([P, H, P], BF16, name="otb", tag="otb", bufs=1)
            ot8 = big.tile([P, H, T - P], F8, name="ot8", tag="ot8", bufs=1)

            chunks = {}

            def load_chunk(tt):
                c = work.tile([P, KD * P], BF16, name=f"xtc{tt}", tag="xtc")
                nc.sync.dma_start(out=c, in_=xtc_d[tt])
                chunks[tt] = c

            load_chunk(0)

            # ---- fill generators (pumped between exp-gated attention ops)
            qk_done = {}

            def qk_fill_gen(h, ws, tiles):
                wq_t, wk_t = ws
                qt_, kt_ = tiles
                for w_t, dst in ((wq_t, qt_), (wk_t, kt_)):
                    for tb in range(TB):
                        ts_ = slice(tb * 512, (tb + 1) * 512)
                        ps = psP.tile([P, 512], F32, name="psfill", tag="psp")
                        for j in range(JD):
                            nc.tensor.matmul(ps,
                                             lhsT=w_t[:, 2 * j:2 * j + 2, :],
                                             rhs=x8p[j][:, :, ts_],
                                             start=(j == 0), stop=(j == JD - 1),
                                             perf_mode=DR)
                            yield
                        nc.vector.tensor_copy(out=dst[:, ts_], in_=ps)
                qk_done[h] = True

            vdone = {}

            def v_gen(tt):
                """V projection for token-tile tt (all heads), token-major:
                x chunk stationary, w_v moving. Prefetches chunk tt+1."""
                if tt + 1 < TT:
                    load_chunk(tt + 1)
                c = chunks[tt]
                for fc in range(F // 512):
                    fs = slice(fc * 512, (fc + 1) * 512)
                    ps = psP.tile([P, 512], F32, name="psv", tag="psp")
                    for kd in range(KD):
                        nc.tensor.matmul(ps, lhsT=c[:, kd * P:(kd + 1) * P],
                                         rhs=wv_sb[:, kd, fs],
                                         start=(kd == 0), stop=(kd == KD - 1))
                        yield
                    nc.vector.tensor_copy(out=vhb[:, tt, fs], in_=ps)
                    nc.vector.tensor_copy(out=vh8[:, tt, fs], in_=ps)
                del chunks[tt]
                vdone[tt] = True

            def alloc_head_tiles(h):
                qt_ = work.tile([P, T], BF16, name=f"qt{h}", tag="qt", bufs=3)
                kt_ = work.tile([P, T], BF16, name=f"kt{h}", tag="kt", bufs=3)
                return qt_, kt_

            def oproj_nb_gen(nb):
                """o_proj chains for token block nb (nb=0 bf16, else fp8 DR).
                Weights prefetched 3 m-tiles ahead so the drain never stalls
                on a weight DMA."""
                PF = 4 if nb else 2
                ns = slice(nb * 512, (nb + 1) * 512)
                ns8 = slice(nb * 512 - P, nb * 512 - P + 512)
                wts = {}

                def load_wt(m):
                    wt8 = work.tile([P, H, P], F8, name=f"wo8_{nb}_{m}",
                                    tag="wo8", bufs=5)
                    nc.sync.dma_start(
                        out=wt8.rearrange("p h f -> p (h f)"), in_=wo8_d[m])
                    if nb == 0:
                        wtb = work.tile([P, H, P], BF16, name=f"wob{m}",
                                        tag="wob", bufs=3)
                        nc.sync.dma_start(
                            out=wtb.rearrange("p h f -> p (h f)"),
                            in_=wob_d[m])
                        wts[m] = (wt8, wtb)
                    else:
                        wts[m] = wt8

                for m in range(PF):
                    load_wt(m)
                for m in range(MT):
                    if m + PF < MT:
                        load_wt(m + PF)
                    wt = wts.pop(m)
                    yield
                    stg = work.tile([P, 512], BF16, name="ostage",
                                    tag="ostage", bufs=3)
                    if nb == 0:
                        # tokens 0-127 bf16 (otb), 128-511 fp8 DR (ot8)
                        wt8, wtb = wt
                        ps1 = psP.tile([P, P], F32, name="psout0", tag="psp")
                        for hh in range(H):
                            nc.tensor.matmul(ps1, lhsT=wtb[:, hh, :],
                                             rhs=otb[:, hh, :],
                                             start=(hh == 0),
                                             stop=(hh == H - 1))
                            yield
                        ps2 = psP.tile([P, 384], F32, name="psout1", tag="psp")
                        for j in range(H // 2):
                            nc.tensor.matmul(ps2,
                                             lhsT=wt8[:, 2 * j:2 * j + 2, :],
                                             rhs=ot8[:, 2 * j:2 * j + 2,
                                                     0:384],
                                             start=(j == 0),
                                             stop=(j == H // 2 - 1),
                                             perf_mode=DR)
                            yield
                        nc.vector.tensor_copy(out=stg[:, :P], in_=ps1)
                        nc.vector.tensor_copy(out=stg[:, P:], in_=ps2)
                    else:
                        ps = psP.tile([P, 512], F32, name="psout", tag="psp")
                        for j in range(H // 2):
                            nc.tensor.matmul(ps, lhsT=wt[:, 2 * j:2 * j + 2, :],
                                             rhs=ot8[:, 2 * j:2 * j + 2, ns8],
                                             start=(j == 0),
                                             stop=(j == H // 2 - 1),
                                             perf_mode=DR)
                            yield
                        nc.vector.tensor_copy(out=stg, in_=ps)
                    nc.sync.dma_start(out=out_d[m * P:(m + 1) * P, ns],
                                      in_=stg)

            def pump(gen, n):
                for _ in range(n):
                    try:
                        next(gen)
                    except StopIteration:
                        return False
                return True

            def pump_n(gen, n):
                c = 0
                for _ in range(n):
                    try:
                        next(gen)
                        c += 1
                    except StopIteration:
                        break
                return c

            fills = []

            def pump_fills(n):
                while n > 0 and fills:
                    n -= pump_n(fills[0], n)
                    if n > 0:
                        fills.pop(0)

            def ensure_v(tt_hi):
                while not vdone.get(tt_hi, False) and fills:
                    pump_fills(16)

            def att_gen(h):
                """Causal attention for head h, yielding at pacing points so
                the driver can interleave TWO heads (each head's S/PV covers
                the other's exp latency) plus fill chains."""
                qt_, kt_ = head_tiles[h]
                hs = slice(h * P, (h + 1) * P)
                for qb in range(TB):
                    if h <= 1:
                        ensure_v(KT_PER_B * (qb + 1) - 1)
                    nkt = KT_PER_B * (qb + 1)
                    off = KT_PER_B * qb  # off-diagonal key tiles (fp8 DR)
                    qs_full = slice(qb * 512, (qb + 1) * 512)
                    # ---- phase A: all S matmuls + exps of this block; P
                    # lands in SBUF (fp8 pairs off-diagonal, bf16 diagonal).
                    # PE work here is light (S only) — fills pumped between
                    # S ops keep it fed while ACT churns through the exps.
                    pt8s = []
                    for j in range(off // 2):
                        pt8_t = work.tile([P, 2, 512], F8, name="pt8",
                                          tag="pt8", bufs=6)
                        pt8s.append(pt8_t)
                        for u in range(2):
                            kt = 2 * j + u
                            psS_t = psS.tile([P, 512], F32, name="pssc",
                                             tag="pss")
                            nc.tensor.matmul(psS_t,
                                             lhsT=kt_[:, kt * P:(kt + 1) * P],
                                             rhs=qt_[:, qs_full],
                                             start=True, stop=True)
                            nc.scalar.activation(out=pt8_t[:, u, :],
                                                 in_=psS_t, func=EXP,
                                                 scale=scale)
                            yield
                    ptd = work.tile([P, KT_PER_B, 512], BF16, name="ptd",
                                    tag="ptd", bufs=2)
                    for di in range(KT_PER_B):
                        kt = off + di
                        c0 = di * P
                        w = 512 - c0
                        qs = slice(qb * 512 + c0, (qb + 1) * 512)
                        psS_t = psS.tile([P, 512], F32, name="pssc", tag="pss")
                        nc.tensor.matmul(psS_t[:, :w],
                                         lhsT=kt_[:, kt * P:(kt + 1) * P],
                                         rhs=qt_[:, qs],
                                         start=True, stop=True)
                        nc.scalar.activation(out=ptd[:, di, :w],
                                             in_=psS_t[:, :w],
                                             func=EXP, scale=scale)
                        nc.vector.tensor_mul(ptd[:, di, :P], ptd[:, di, :P],
                                             cmask)
                        yield
                    # ---- phase B: PV + row-sum matmuls (dense PE, no exp
                    # dependency nearer than a full phase — ACT backlog can't
                    # stall it; the paired head's phase A overlaps here)
                    psO_t = psO.tile([P, 512], F32, name="psodt", tag="pso")
                    psR_t = (psR.tile([P, 512], F32, name="psrow", tag="psr")
                             if off > 0 else None)
                    for j in range(off // 2):
                        pt8_t = pt8s[j]
                        nc.tensor.matmul(psO_t,
                                         lhsT=vh8[:, 2 * j:2 * j + 2, hs],
                                         rhs=pt8_t,
                                         start=(j == 0), stop=False,
                                         perf_mode=DR, skip_group_check=True)
                        nc.tensor.matmul(psR_t, lhsT=ones8, rhs=pt8_t,
                                         start=(j == 0), stop=False,
                                         perf_mode=DR, skip_group_check=True)
                        yield
                    racc = work.tile([P, 512], F32, name="racc", tag="racc",
                                     bufs=2)
                    for di in range(KT_PER_B):
                        kt = off + di
                        c0 = di * P
                        w = 512 - c0
                        nc.tensor.matmul(psO_t[:, c0:], lhsT=vhb[:, kt, hs],
                                         rhs=ptd[:, di, :w],
                                         start=(kt == 0),
                                         stop=(kt == nkt - 1),
                                         skip_group_check=True)
                        if di == 0:
                            nc.vector.tensor_copy(out=racc, in_=ptd[:, 0, :])
                        else:
                            nc.vector.tensor_add(racc[:, c0:], racc[:, c0:],
                                                 ptd[:, di, :w])
                        if di & 1:
                            yield
                    # cross-partition reduce the diagonal partials into psR
                    # (one ones-matmul per block; closes the psR group)
                    raccb = work.tile([P, 512], BF16, name="raccb",
                                      tag="raccb")
                    nc.vector.tensor_copy(out=raccb, in_=racc)
                    if psR_t is None:
                        psR_t = psR.tile([P, 512], F32, name="psrow",
                                         tag="psr")
                    nc.tensor.matmul(psR_t, lhsT=ones, rhs=raccb,
                                     start=(off == 0), stop=True,
                                     skip_group_check=True)
                    nc.vector.reciprocal_approx_fast(out=racc, in_=psR_t)
                    if qb == 0:
                        nc.vector.tensor_mul(otb[:, h, :], psO_t[:, :P],
                                             racc[:, :P])
                        nc.vector.tensor_mul(ot8[:, h, 0:384], psO_t[:, P:],
                                             racc[:, P:])
                    else:
                        nc.vector.tensor_mul(
                            ot8[:, h, qb * 512 - P:qb * 512 - P + 512],
                            psO_t, racc)
                    if h == H - 1:
                        fills.append(oproj_nb_gen(qb))
                    yield

            # head-0 Q/K projections up front, kd-pair-OUTER across all 8
            # PSUM banks: the first matmul needs only the first x8 slice, so
            # the PE starts ~2us after the x8 DMAs begin instead of waiting
            # for the whole 4MB transfer
            head_tiles = [None] * H
            head_tiles[0] = alloc_head_tiles(0)
            qt0, kt0 = head_tiles[0]
            wq0, wk0 = ws_list[0]
            qpools = [(psP, "psp"), (psP, "psp"), (psS, "pss"),
                      (psS, "pss")][:TB]
            kpools = [(psO, "pso"), (psO, "pso"), (psR, "psr"),
                      (psR, "psr")][:TB]
            psq = [pool.tile([P, 512], F32, name=f"psq{t}", tag=tag)
                   for t, (pool, tag) in enumerate(qpools)]
            psk = [pool.tile([P, 512], F32, name=f"psk{t}", tag=tag)
                   for t, (pool, tag) in enumerate(kpools)]
            for j in range(JD):
                for tb in range(TB):
                    ts_ = slice(tb * 512, (tb + 1) * 512)
                    nc.tensor.matmul(psq[tb], lhsT=wq0[:, 2 * j:2 * j + 2, :],
                                     rhs=x8p[j][:, :, ts_],
                                     start=(j == 0), stop=(j == JD - 1),
                                     perf_mode=DR, skip_group_check=True)
                    nc.tensor.matmul(psk[tb], lhsT=wk0[:, 2 * j:2 * j + 2, :],
                                     rhs=x8p[j][:, :, ts_],
                                     start=(j == 0), stop=(j == JD - 1),
                                     perf_mode=DR, skip_group_check=True)
            for tb in range(TB):
                ts_ = slice(tb * 512, (tb + 1) * 512)
                nc.vector.tensor_copy(out=qt0[:, ts_], in_=psq[tb])
                nc.vector.tensor_copy(out=kt0[:, ts_], in_=psk[tb])
            qk_done[0] = True

            # fills: qk(1) first (head 1 activates early under the staggered
            # pairing), then the V chains, then later heads' qk as they queue
            head_tiles[1] = alloc_head_tiles(1)
            fills.append(qk_fill_gen(1, ws_list[1], head_tiles[1]))
            qk_queued = 2
            for tt in range(TT):
                fills.append(v_gen(tt))

            def queue_qk_through(h_hi):
                nonlocal qk_queued
                while qk_queued < min(h_hi + 1, H):
                    hq = qk_queued
                    if hq + 1 < H and ws_list[hq + 1] is None:
                        ws_list[hq + 1] = load_qk_weights(hq + 1)
                    head_tiles[hq] = alloc_head_tiles(hq)
                    fills.append(qk_fill_gen(hq, ws_list[hq], head_tiles[hq]))
                    qk_queued += 1

            active = []
            next_h = 0
            while active or next_h < H:
                if len(active) < 2 and next_h < H:
                    h = next_h
                    queue_qk_through(h + 2)
                    if h > 0:
                        while not qk_done.get(h, False) and fills:
                            pump_fills(16)
                    active.append(att_gen(h))
                    next_h += 1
                for g in list(active):
                    try:
                        next(g)
                    except StopIteration:
                        active.remove(g)
                pump_fills(3)
            while fills:
                pump_fills(64)

    nc.compile()
    return nc


def _ternary(w):
    """BitLinear ternary weights + gamma: clip(round(w/gamma),-1,1), gamma."""
    w = np.asarray(w, dtype=np.float32)
    gamma = max(np.float32(np.abs(w).mean(dtype=np.float32)), np.float32(1e-5))
    q = np.clip(np.round(w / gamma), -1.0, 1.0).astype(np.float32)
    return q, gamma


def _causal_mask():
    k = np.arange(128)[:, None]
    q = np.arange(128)[None, :]
    return (k <= q).astype(np.float32).astype(ml_dtypes.bfloat16)


def _tile_qkv(t_shard):
    """[F, D] -> [H, 128, KD*128]: [h, p, kd*128+f] = t[h*128+f, kd*128+p]."""
    Fs, Ds = t_shard.shape
    a = t_shard.reshape(Fs // 128, 128, Ds // 128, 128)  # [h, f, kd, p]
    a = a.transpose(0, 3, 2, 1).reshape(Fs // 128, 128, Ds)
    return np.ascontiguousarray(a)


def _tile_wo(t_shard):
    """[D, F] -> [MT, 128, H*128]: [m, p, h*128+j] = t[m*128+j, h*128+p]."""
    Ds, Fs = t_shard.shape
    a = t_shard.reshape(Ds // 128, 128, Fs // 128, 128)  # [m, j, h, p]
    a = a.transpose(0, 3, 2, 1).reshape(Ds // 128, 128, Fs)
    return np.ascontiguousarray(a)


def _tile_xtc(xb):
    """[T, D] -> [TT, 128, KD*128]: [tt, p, kd*128+j] = x[tt*128+j, kd*128+p]."""
    T, D = xb.shape
    a = xb.reshape(T // 128, 128, D // 128, 128)  # [tt, j, kd, p]
    a = a.transpose(0, 3, 2, 1).reshape(T // 128, 128, D)
    return np.ascontiguousarray(a)


def _prep_inputs(x, wq, wk, wv, wo):
    bf = ml_dtypes.bfloat16
    f8 = ml_dtypes.float8_e4m3
    x = np.asarray(x, dtype=np.float32)
    tq, gq = _ternary(wq)
    tk, gk = _ternary(wk)
    tv, gv = _ternary(wv)
    to, go = _ternary(wo)
    scale = float(gq) * float(gk) / math.sqrt(D_HEAD)
    oscale = 2.0 * float(gv) * float(go)
    cmask = _causal_mask()

    def _part_major(a):  # [D, N] -> [128, (D//128)*N], [p, kd*N+n] = a[kd*128+p, n]
        D_, N_ = a.shape
        return np.ascontiguousarray(
            a.reshape(D_ // 128, 128, N_).transpose(1, 0, 2).reshape(128, -1))

    x8s = [_part_major(np.ascontiguousarray(x[b].T)).astype(f8)
           for b in range(B)]
    xtcs = [_tile_xtc(x[b]).astype(bf) for b in range(B)]
    shards = []
    for hg in range(2):
        rows = slice(hg * F_LOC, (hg + 1) * F_LOC)
        wvm = _part_major((0.5 * tv[rows, :]).T)
        shards.append({
            "wq8": _tile_qkv(tq[rows, :]).astype(f8),
            "wk8": _tile_qkv(tk[rows, :]).astype(f8),
            "wvm": wvm.astype(bf),
            "wo8": _tile_wo(to[:, rows]).astype(f8),
            "wob": _tile_wo(to[:, rows]).astype(bf),
        })
    in_maps = []
    for c in range(N_CORES):
        b, hg = c // 2, c % 2
        m = {"x8": x8s[b], "xtc": xtcs[b], "cmask": cmask}
        m.update(shards[hg])
        in_maps.append(m)
    return in_maps, scale, oscale


_NC_CACHE = {}


def _get_nc(scale):
    key = round(float(scale), 12)
    if key not in _NC_CACHE:
        _NC_CACHE[key] = build_bass(scale)
    return _NC_CACHE[key]


def run(x, wq, wk, wv, wo, trace=False):
    in_maps, scale, oscale = _prep_inputs(x, wq, wk, wv, wo)
    nc = _get_nc(scale)
    res = bass_utils.run_bass_kernel_spmd(
        nc, in_maps, core_ids=list(range(N_CORES)), trace=trace)
    out = np.empty((B, T_FULL, D_MODEL), dtype=np.float32)
    for b in range(B):
        a = np.asarray(res.results[2 * b]["outT"], dtype=np.float32)
        c = np.asarray(res.results[2 * b + 1]["outT"], dtype=np.float32)
        out[b] = (a + c).T * oscale
    return out, res


def kernel(x, wq, wk, wv, wo):
    out, _ = run(x, wq, wk, wv, wo)
    return out


# revision 40
# speedup vs baseline: 1.0040x; 1.0040x over previous
"""BitSelfAttention on 8 TRN2 NeuronCores — fp8 DoubleRow hybrid.

Sharding: core c handles batch b = c//2 and head-group hg = c%2 (8 of 16
heads). Each core computes its 8 heads' QKV projections + causal attention +
its slice of the o_proj GEMM, producing a partial output ([D, T], fp32);
host sums the two head-group partials per batch and applies the folded
BitLinear gammas.

The device works in pure-ternary units: BitLinear weights are sent as their
ternary {-1,0,+1} values (EXACT in fp8e4), gamma_q*gamma_k is folded into the
softmax exp scale, and 2*gamma_v*gamma_o is applied on the host (the V
weights carry an extra 0.5 — exact in bf16 — so that |v~| stays < 240, the
TRN fp8e4 saturation point).

fp8 DoubleRow (2 contraction subtiles per MM, ~1.8x MAC throughput) is used
where a numpy error simulation showed it is safe (sim rel err 0.92% vs the
2e-2 gate; all-fp8 variants fail):
  - Q/K projections:  fp8 DR (x as e4m3 moving operand, ternary w stationary)
  - V projection:     bf16, x STATIONARY per token-tile so V lands
                      token-major directly — kills the 128 PE transposes the
                      baseline spent ~35us on. w_v is the moving operand.
  - QK^T scores:      bf16 (contraction is d_head=128 — DR inapplicable)
  - P@V off-diagonal: fp8 DR (exp emits e4m3, V kept in an fp8 copy)
  - P@V diagonal:     bf16 (peaked early-token rows need accurate V)
  - o_proj:           tokens 0-127 bf16 (the absmax-setting peaked rows),
                      tokens 128+ fp8 DR
Scheduling: TWO heads' attention runs staggered, each block split into
phase A (all S matmuls + exps; P parked in SBUF) and phase B (all PV /
row-sum matmuls) so a backlogged activation engine can never stall the PE;
projection/V/o_proj fill chains are pumped between steps. Off-diagonal
row-sums ride the PE (fp8-DR all-ones stationary into a PSUM bank);
diagonal partials accumulate on DVE and one ones-matmul per block does the
cross-partition reduce+broadcast; normalization via fast reciprocal.
x8 is loaded as one tile per DR pair so the first projection matmul starts
after ~1/8 of the transfer; outputs stream back as bf16.
"""

import math

import ml_dtypes
import numpy as np

import concourse.mybir as mybir
import concourse.tile as tile
from concourse import bacc
from concourse import bass_utils

BF16 = mybir.dt.bfloat16
F32 = mybir.dt.float32
F8 = mybir.dt.float8e4
DR = mybir.MatmulPerfMode.DoubleRow
EXP = mybir.ActivationFunctionType.Exp

D_MODEL = 2048
N_HEAD = 16
D_HEAD = 128
B = 4
T_FULL = 2048
N_CORES = 8
F_LOC = D_MODEL // 2  # features per core (8 heads)


def build_bass(scale, T=T_FULL, D=D_MODEL, F=F_LOC, debug=False):
    """Build the single-core program (SPMD across 8 cores via input data)."""
    P = 128
    KD = D // P      # contraction 128-tiles (16)
    JD = KD // 2     # DR pairs over contraction (8)
    TT = T // P      # token 128-tiles
    TB = T // 512    # token 512-blocks
    H = F // P       # local heads (8)
    MT = D // P      # output-dmodel 128-tiles (16)
    KT_PER_B = 512 // P

    nc = bacc.Bacc("TRN2", target_bir_lowering=False, debug=debug,
                   num_devices=N_CORES)
    # x8/wvm arrive pre-arranged in their exact SBUF layouts (single DMA)
    x8_d = nc.dram_tensor("x8", [P, KD * T], F8, kind="ExternalInput").ap()
    xtc_d = nc.dram_tensor("xtc", [TT, P, KD * P], BF16,
                           kind="ExternalInput").ap()
    # wq8/wk8: [H, 128, KD*128] ternary fp8, [h, p, kd*128+f] = t[h*128+f, kd*128+p]
    wq8_d = nc.dram_tensor("wq8", [H, P, KD * P], F8, kind="ExternalInput").ap()
    wk8_d = nc.dram_tensor("wk8", [H, P, KD * P], F8, kind="ExternalInput").ap()
    # wvm: [128, KD*F] bf16, [p, kd*F+f] = 0.5 * tv[f, kd*128+p] (moving op)
    wvm_d = nc.dram_tensor("wvm", [P, KD * F], BF16, kind="ExternalInput").ap()
    # wo: [MT, 128, H*128], [m, p, h*128+j] = to[m*128+j, h*128+p]
    wo8_d = nc.dram_tensor("wo8", [MT, P, H * P], F8, kind="ExternalInput").ap()
    wob_d = nc.dram_tensor("wob", [MT, P, H * P], BF16,
                           kind="ExternalInput").ap()
    cm_d = nc.dram_tensor("cmask", [P, P], BF16, kind="ExternalInput").ap()
    out_d = nc.dram_tensor("outT", [D, T], BF16, kind="ExternalOutput").ap()

    with tile.TileContext(nc) as tc:
        with (
            tc.tile_pool(name="big", bufs=1) as big,
            tc.tile_pool(name="work", bufs=2) as work,
            tc.tile_pool(name="psS", bufs=2, space="PSUM") as psS,
            tc.tile_pool(name="psO", bufs=2, space="PSUM") as psO,
            tc.tile_pool(name="psR", bufs=2, space="PSUM") as psR,
            tc.tile_pool(name="psP", bufs=2, space="PSUM") as psP,
        ):
            # ---- persistent inputs (head-0 weights first: first MMs need them)
            def load_qk_weights(h):
                wq_t = work.tile([P, KD, P], F8, name=f"wq{h}", tag="wq8",
                                 bufs=3)
                nc.sync.dma_start(out=wq_t.rearrange("p kd f -> p (kd f)"),
                                  in_=wq8_d[h])
                wk_t = work.tile([P, KD, P], F8, name=f"wk{h}", tag="wk8",
                                 bufs=3)
                nc.sync.dma_start(out=wk_t.rearrange("p kd f -> p (kd f)"),
                                  in_=wk8_d[h])
                return wq_t, wk_t

            ws_list = [None] * (H + 3)
            ws_list[0] = load_qk_weights(0)
            # x8 as one tile per DR kd-pair: dependencies stay per-pair, so
            # the first projection matmul starts after ~1/8 of the transfer
            x8p = [big.tile([P, 2, T], F8, name=f"x8p{j}", tag="x8p", bufs=JD)
                   for j in range(JD)]
            for j in range(JD):
                nc.sync.dma_start(
                    out=x8p[j].rearrange("p a t -> p (a t)"),
                    in_=x8_d[:, 2 * j * T:(2 * j + 2) * T])
            if H > 1:
                ws_list[1] = load_qk_weights(1)
            if H > 2:
                ws_list[2] = load_qk_weights(2)
            wv_sb = big.tile([P, KD, F], BF16, name="wv_sb", tag="wv", bufs=1)
            nc.sync.dma_start(
                out=wv_sb.rearrange("p kd f -> p (kd f)"), in_=wvm_d)
            ones = big.tile([P, P], BF16, name="ones_sb", tag="ones", bufs=1)
            nc.vector.memset(ones, 1.0)
            ones8 = big.tile([P, 2, P], F8, name="ones8_sb", tag="ones8",
                             bufs=1)
            nc.vector.memset(ones8, 1.0)
            cmask = big.tile([P, P], BF16, name="cmask_sb", tag="cmask", bufs=1)
            nc.sync.dma_start(out=cmask, in_=cm_d)
            vhb = big.tile([P, TT, F], BF16, name="vhb", tag="vhb", bufs=1)
            vh8 = big.tile([P, TT, F], F8, name="vh8", tag="vh8", bufs=1)
            # only tokens 0-127 (the sharply peaked rows) keep a bf16 o_proj
            # path; everything else consumes the fp8 copy
            otb = big.tile([P, H, P], BF16, name="otb", tag="otb", bufs=1)
            ot8 = big.tile([P, H, T - P], F8, name="ot8", tag="ot8", bufs=1)

            chunks = {}

            def load_chunk(tt):
                c = work.tile([P, KD * P], BF16, name=f"xtc{tt}", tag="xtc")
                nc.sync.dma_start(out=c, in_=xtc_d[tt])
                chunks[tt] = c

            load_chunk(0)

            # ---- fill generators (pumped between exp-gated attention ops)
            qk_done = {}

            def qk_fill_gen(h, ws, tiles):
                wq_t, wk_t = ws
                qt_, kt_ = tiles
                for w_t, dst in ((wq_t, qt_), (wk_t, kt_)):
                    for tb in range(TB):
                        ts_ = slice(tb * 512, (tb + 1) * 512)
                        ps = psP.tile([P, 512], F32, name="psfill", tag="psp")
                        for j in range(JD):
                            nc.tensor.matmul(ps,
                                             lhsT=w_t[:, 2 * j:2 * j + 2, :],
                                             rhs=x8p[j][:, :, ts_],
                                             start=(j == 0), stop=(j == JD - 1),
                                             perf_mode=DR)
                            yield
                        nc.vector.tensor_copy(out=dst[:, ts_], in_=ps)
                qk_done[h] = True

            vdone = {}

            def v_gen(tt):
                """V projection for token-tile tt (all heads), token-major:
                x chunk stationary, w_v moving. Prefetches chunk tt+1."""
                if tt + 1 < TT:
                    load_chunk(tt + 1)
                c = chunks[tt]
                for fc in range(F // 512):
                    fs = slice(fc * 512, (fc + 1) * 512)
                    ps = psP.tile([P, 512], F32, name="psv", tag="psp")
                    for kd in range(KD):
                        nc.tensor.matmul(ps, lhsT=c[:, kd * P:(kd + 1) * P],
                                         rhs=wv_sb[:, kd, fs],
                                         start=(kd == 0), stop=(kd == KD - 1))
                        yield
                    nc.vector.tensor_copy(out=vhb[:, tt, fs], in_=ps)
                    nc.vector.tensor_copy(out=vh8[:, tt, fs], in_=ps)
                del chunks[tt]
                vdone[tt] = True

            def alloc_head_tiles(h):
                qt_ = work.tile([P, T], BF16, name=f"qt{h}", tag="qt", bufs=3)
                kt_ = work.tile([P, T], BF16, name=f"kt{h}", tag="kt", bufs=3)
                return qt_, kt_

            def oproj_nb_gen(nb):
                """o_proj chains for token block nb (nb=0 bf16, else fp8 DR).
                Weights prefetched 3 m-tiles ahead so the drain never stalls
                on a weight DMA."""
                PF = 4 if nb else 2
                ns = slice(nb * 512, (nb + 1) * 512)
                ns8 = slice(nb * 512 - P, nb * 512 - P + 512)
                wts = {}

                def load_wt(m):
                    wt8 = work.tile([P, H, P], F8, name=f"wo8_{nb}_{m}",
                                    tag="wo8", bufs=5)
                    nc.sync.dma_start(
                        out=wt8.rearrange("p h f -> p (h f)"), in_=wo8_d[m])
                    if nb == 0:
                        wtb = work.tile([P, H, P], BF16, name=f"wob{m}",
                                        tag="wob", bufs=3)
                        nc.sync.dma_start(
                            out=wtb.rearrange("p h f -> p (h f)"),
                            in_=wob_d[m])
                        wts[m] = (wt8, wtb)
                    else:
                        wts[m] = wt8

                for m in range(PF):
                    load_wt(m)
                for m in range(MT):
                    if m + PF < MT:
                        load_wt(m + PF)
                    wt = wts.pop(m)
                    yield
                    stg = work.tile([P, 512], BF16, name="ostage",
                                    tag="ostage", bufs=3)
                    if nb == 0:
                        # tokens 0-127 bf16 (otb), 128-511 fp8 DR (ot8)
                        wt8, wtb = wt
                        ps1 = psP.tile([P, P], F32, name="psout0", tag="psp")
                        for hh in range(H):
                            nc.tensor.matmul(ps1, lhsT=wtb[:, hh, :],
                                             rhs=otb[:, hh, :],
                                             start=(hh == 0),
                                             stop=(hh == H - 1))
                            yield
                        ps2 = psP.tile([P, 384], F32, name="psout1", tag="psp")
                        for j in range(H // 2):
                            nc.tensor.matmul(ps2,
                                             lhsT=wt8[:, 2 * j:2 * j + 2, :],
                                             rhs=ot8[:, 2 * j:2 * j + 2,
                                                     0:384],
                                             start=(j == 0),
                                             stop=(j == H // 2 - 1),
                                             perf_mode=DR)
                            yield
                        nc.vector.tensor_copy(out=stg[:, :P], in_=ps1)
                        nc.vector.tensor_copy(out=stg[:, P:], in_=ps2)
                    else:
                        ps = psP.tile([P, 512], F32, name="psout", tag="psp")
                        for j in range(H // 2):
                            nc.tensor.matmul(ps, lhsT=wt[:, 2 * j:2 * j + 2, :],
                                             rhs=ot8[:, 2 * j:2 * j + 2, ns8],
                                             start=(j == 0),
                                             stop=(j == H // 2 - 1),
                                             perf_mode=DR)
                            yield
                        nc.vector.tensor_copy(out=stg, in_=ps)
                    nc.sync.dma_start(out=out_d[m * P:(m + 1) * P, ns],
                                      in_=stg)

            def pump(gen, n):
                for _ in range(n):
                    try:
                        next(gen)
                    except StopIteration:
                        return False
                return True

            def pump_n(gen, n):
                c = 0
                for _ in range(n):
                    try:
                        next(gen)
                        c += 1
                    except StopIteration:
                        break
                return c

            fills = []

            def pump_fills(n):
                while n > 0 and fills:
                    n -= pump_n(fills[0], n)
                    if n > 0:
                        fills.pop(0)

            def ensure_v(tt_hi):
                while not vdone.get(tt_hi, False) and fills:
                    pump_fills(16)

            def att_gen(h):
                """Causal attention for head h, yielding at pacing points so
                the driver can interleave TWO heads (each head's S/PV covers
                the other's exp latency) plus fill chains."""
                qt_, kt_ = head_tiles[h]
                hs = slice(h * P, (h + 1) * P)
                for qb in range(TB):
                    if h <= 1:
                        ensure_v(KT_PER_B * (qb + 1) - 1)
                    nkt = KT_PER_B * (qb + 1)
                    off = KT_PER_B * qb  # off-diagonal key tiles (fp8 DR)
                    qs_full = slice(qb * 512, (qb + 1) * 512)
                    # ---- phase A: all S matmuls + exps of this block; P
                    # lands in SBUF (fp8 pairs off-diagonal, bf16 diagonal).
                    # PE work here is light (S only) — fills pumped between
                    # S ops keep it fed while ACT churns through the exps.
                    pt8s = []
                    for j in range(off // 2):
                        pt8_t = work.tile([P, 2, 512], F8, name="pt8",
                                          tag="pt8", bufs=6)
                        pt8s.append(pt8_t)
                        for u in range(2):
                            kt = 2 * j + u
                            psS_t = psS.tile([P, 512], F32, name="pssc",
                                             tag="pss")
                            nc.tensor.matmul(psS_t,
                                             lhsT=kt_[:, kt * P:(kt + 1) * P],
                                             rhs=qt_[:, qs_full],
                                             start=True, stop=True)
                            nc.scalar.activation(out=pt8_t[:, u, :],
                                                 in_=psS_t, func=EXP,
                                                 scale=scale)
                            yield
                    ptd = work.tile([P, KT_PER_B, 512], BF16, name="ptd",
                                    tag="ptd", bufs=2)
                    for di in range(KT_PER_B):
                        kt = off + di
                        c0 = di * P
                        w = 512 - c0
                        qs = slice(qb * 512 + c0, (qb + 1) * 512)
                        psS_t = psS.tile([P, 512], F32, name="pssc", tag="pss")
                        nc.tensor.matmul(psS_t[:, :w],
                                         lhsT=kt_[:, kt * P:(kt + 1) * P],
                                         rhs=qt_[:, qs],
                                         start=True, stop=True)
                        nc.scalar.activation(out=ptd[:, di, :w],
                                             in_=psS_t[:, :w],
                                             func=EXP, scale=scale)
                        nc.vector.tensor_mul(ptd[:, di, :P], ptd[:, di, :P],
                                             cmask)
                        yield
                    # ---- phase B: PV + row-sum matmuls (dense PE, no exp
                    # dependency nearer than a full phase — ACT backlog can't
                    # stall it; the paired head's phase A overlaps here)
                    psO_t = psO.tile([P, 512], F32, name="psodt", tag="pso")
                    psR_t = (psR.tile([P, 512], F32, name="psrow", tag="psr")
                             if off > 0 else None)
                    for j in range(off // 2):
                        pt8_t = pt8s[j]
                        nc.tensor.matmul(psO_t,
                                         lhsT=vh8[:, 2 * j:2 * j + 2, hs],
                                         rhs=pt8_t,
                                         start=(j == 0), stop=False,
                                         perf_mode=DR, skip_group_check=True)
                        nc.tensor.matmul(psR_t, lhsT=ones8, rhs=pt8_t,
                                         start=(j == 0), stop=False,
                                         perf_mode=DR, skip_group_check=True)
                        yield
                    racc = work.tile([P, 512], F32, name="racc", tag="racc",
                                     bufs=2)
                    for di in range(KT_PER_B):
                        kt = off + di
                        c0 = di * P
                        w = 512 - c0
                        nc.tensor.matmul(psO_t[:, c0:], lhsT=vhb[:, kt, hs],
                                         rhs=ptd[:, di, :w],
                                         start=(kt == 0),
                                         stop=(kt == nkt - 1),
                                         skip_group_check=True)
                        if di == 0:
                            nc.vector.tensor_copy(out=racc, in_=ptd[:, 0, :])
                        else:
                            nc.vector.tensor_add(racc[:, c0:], racc[:, c0:],
                                                 ptd[:, di, :w])
                        if di & 1:
                            yield
                    # cross-partition reduce the diagonal partials into psR
                    # (one ones-matmul per block; closes the psR group)
                    raccb = work.tile([P, 512], BF16, name="raccb",
                                      tag="raccb")
                    nc.vector.tensor_copy(out=raccb, in_=racc)
                    if psR_t is None:
                        psR_t = psR.tile([P, 512], F32, name="psrow",
                                         tag="psr")
                    nc.tensor.matmul(psR_t, lhsT=ones, rhs=raccb,
                                     start=(off == 0), stop=True,
                                     skip_group_check=True)
                    nc.vector.reciprocal_approx_fast(out=racc, in_=psR_t)
                    if qb == 0:
                        nc.vector.tensor_mul(otb[:, h, :], psO_t[:, :P],
                                             racc[:, :P])
                        nc.vector.tensor_mul(ot8[:, h, 0:384], psO_t[:, P:],
                                             racc[:, P:])
                    else:
                        nc.vector.tensor_mul(
                            ot8[:, h, qb * 512 - P:qb * 512 - P + 512],
                            psO_t, racc)
                    if h == H - 1:
                        fills.append(oproj_nb_gen(qb))
                    yield

            # head-0 Q/K projections up front, kd-pair-OUTER across all 8
            # PSUM banks: the first matmul needs only the first x8 slice, so
            # the PE starts ~2us after the x8 DMAs begin instead of waiting
            # for the whole 4MB transfer
            head_tiles = [None] * H
            head_tiles[0] = alloc_head_tiles(0)
            qt0, kt0 = head_tiles[0]
            wq0, wk0 = ws_list[0]
            qpools = [(psP, "psp"), (psP, "psp"), (psS, "pss"),
                      (psS, "pss")][:TB]
            kpools = [(psO, "pso"), (psO, "pso"), (psR, "psr"),
                      (psR, "psr")][:TB]
            psq = [pool.tile([P, 512], F32, name=f"psq{t}", tag=tag)
                   for t, (pool, tag) in enumerate(qpools)]
            psk = [pool.tile([P, 512], F32, name=f"psk{t}", tag=tag)
                   for t, (pool, tag) in enumerate(kpools)]
            for j in range(JD):
                for tb in range(TB):
                    ts_ = slice(tb * 512, (tb + 1) * 512)
                    nc.tensor.matmul(psq[tb], lhsT=wq0[:, 2 * j:2 * j + 2, :],
                                     rhs=x8p[j][:, :, ts_],
                                     start=(j == 0), stop=(j == JD - 1),
                                     perf_mode=DR, skip_group_check=True)
                    nc.tensor.matmul(psk[tb], lhsT=wk0[:, 2 * j:2 * j + 2, :],
                                     rhs=x8p[j][:, :, ts_],
                                     start=(j == 0), stop=(j == JD - 1),
                                     perf_mode=DR, skip_group_check=True)
            for tb in range(TB):
                ts_ = slice(tb * 512, (tb + 1) * 512)
                nc.vector.tensor_copy(out=qt0[:, ts_], in_=psq[tb])
                nc.vector.tensor_copy(out=kt0[:, ts_], in_=psk[tb])
            qk_done[0] = True

            # fills: qk(1) first (head 1 activates early under the staggered
            # pairing), then the V chains, then later heads' qk as they queue
            head_tiles[1] = alloc_head_tiles(1)
            fills.append(qk_fill_gen(1, ws_list[1], head_tiles[1]))
            qk_queued = 2
            for tt in range(TT):
                fills.append(v_gen(tt))

            def queue_qk_through(h_hi):
                nonlocal qk_queued
                while qk_queued < min(h_hi + 1, H):
                    hq = qk_queued
                    if hq + 1 < H and ws_list[hq + 1] is None:
                        ws_list[hq + 1] = load_qk_weights(hq + 1)
                    head_tiles[hq] = alloc_head_tiles(hq)
                    fills.append(qk_fill_gen(hq, ws_list[hq], head_tiles[hq]))
                    qk_queued += 1

            active = []
            next_h = 0
            while active or next_h < H:
                if len(active) < 2 and next_h < H:
                    h = next_h
                    queue_qk_through(h + 2)
                    if h > 0:
                        while not qk_done.get(h, False) and fills:
                            pump_fills(16)
                    active.append(att_gen(h))
                    next_h += 1
                for g in list(active):
                    try:
                        next(g)
                    except StopIteration:
                        active.remove(g)
                # solo head (start/end of the stagger) has half the chain
                # work per round — pump fills harder so o_proj keeps the PE
                # fed through the final head's exp-heavy stretch
                pump_fills(3 if len(active) >= 2 else 7)
            while fills:
                pump_fills(64)

    nc.compile()
    return nc


def _ternary(w):
    """BitLinear ternary weights + gamma: clip(round(w/gamma),-1,1), gamma."""
    w = np.asarray(w, dtype=np.float32)
    gamma = max(np.float32(np.abs(w).mean(dtype=np.float32)), np.float32(1e-5))
    q = np.clip(np.round(w / gamma), -1.0, 1.0).astype(np.float32)
    return q, gamma


def _causal_mask():
    k = np.arange(128)[:, None]
    q = np.arange(128)[None, :]
    return (k <= q).astype(np.float32).astype(ml_dtypes.bfloat16)


def _tile_qkv(t_shard):
    """[F, D] -> [H, 128, KD*128]: [h, p, kd*128+f] = t[h*128+f, kd*128+p]."""
    Fs, Ds = t_shard.shape
    a = t_shard.reshape(Fs // 128, 128, Ds // 128, 128)  # [h, f, kd, p]
    a = a.transpose(0, 3, 2, 1).reshape(Fs // 128, 128, Ds)
    return np.ascontiguousarray(a)


def _tile_wo(t_shard):
    """[D, F] -> [MT, 128, H*128]: [m, p, h*128+j] = t[m*128+j, h*128+p]."""
    Ds, Fs = t_shard.shape
    a = t_shard.reshape(Ds // 128, 128, Fs // 128, 128)  # [m, j, h, p]
    a = a.transpose(0, 3, 2, 1).reshape(Ds // 128, 128, Fs)
    return np.ascontiguousarray(a)


def _tile_xtc(xb):
    """[T, D] -> [TT, 128, KD*128]: [tt, p, kd*128+j] = x[tt*128+j, kd*128+p]."""
    T, D = xb.shape
    a = xb.reshape(T // 128, 128, D // 128, 128)  # [tt, j, kd, p]
    a = a.transpose(0, 3, 2, 1).reshape(T // 128, 128, D)
    return np.ascontiguousarray(a)


def _prep_inputs(x, wq, wk, wv, wo):
    bf = ml_dtypes.bfloat16
    f8 = ml_dtypes.float8_e4m3
    x = np.asarray(x, dtype=np.float32)
    tq, gq = _ternary(wq)
    tk, gk = _ternary(wk)
    tv, gv = _ternary(wv)
    to, go = _ternary(wo)
    scale = float(gq) * float(gk) / math.sqrt(D_HEAD)
    oscale = 2.0 * float(gv) * float(go)
    cmask = _causal_mask()

    def _part_major(a):  # [D, N] -> [128, (D//128)*N], [p, kd*N+n] = a[kd*128+p, n]
        D_, N_ = a.shape
        return np.ascontiguousarray(
            a.reshape(D_ // 128, 128, N_).transpose(1, 0, 2).reshape(128, -1))

    x8s = [_part_major(np.ascontiguousarray(x[b].T)).astype(f8)
           for b in range(B)]
    xtcs = [_tile_xtc(x[b]).astype(bf) for b in range(B)]
    shards = []
    for hg in range(2):
        rows = slice(hg * F_LOC, (hg + 1) * F_LOC)
        wvm = _part_major((0.5 * tv[rows, :]).T)
        shards.append({
            "wq8": _tile_qkv(tq[rows, :]).astype(f8),
            "wk8": _tile_qkv(tk[rows, :]).astype(f8),
            "wvm": wvm.astype(bf),
            "wo8": _tile_wo(to[:, rows]).astype(f8),
            "wob": _tile_wo(to[:, rows]).astype(bf),
        })
    in_maps = []
    for c in range(N_CORES):
        b, hg = c // 2, c % 2
        m = {"x8": x8s[b], "xtc": xtcs[b], "cmask": cmask}
        m.update(shards[hg])
        in_maps.append(m)
    return in_maps, scale, oscale


_NC_CACHE = {}


def _get_nc(scale):
    key = round(float(scale), 12)
    if key not in _NC_CACHE:
        _NC_CACHE[key] = build_bass(scale)
    return _NC_CACHE[key]


def run(x, wq, wk, wv, wo, trace=False):
    in_maps, scale, oscale = _prep_inputs(x, wq, wk, wv, wo)
    nc = _get_nc(scale)
    res = bass_utils.run_bass_kernel_spmd(
        nc, in_maps, core_ids=list(range(N_CORES)), trace=trace)
    out = np.empty((B, T_FULL, D_MODEL), dtype=np.float32)
    for b in range(B):
        a = np.asarray(res.results[2 * b]["outT"], dtype=np.float32)
        c = np.asarray(res.results[2 * b + 1]["outT"], dtype=np.float32)
        out[b] = (a + c).T * oscale
    return out, res


def kernel(x, wq, wk, wv, wo):
    out, _ = run(x, wq, wk, wv, wo)
    return out


# revision 41
# speedup vs baseline: 1.0204x; 1.0163x over previous
"""BitSelfAttention on 8 TRN2 NeuronCores — fp8 DoubleRow hybrid.

Sharding: core c handles batch b = c//2 and head-group hg = c%2 (8 of 16
heads). Each core computes its 8 heads' QKV projections + causal attention +
its slice of the o_proj GEMM, producing a partial output ([D, T], fp32);
host sums the two head-group partials per batch and applies the folded
BitLinear gammas.

The device works in pure-ternary units: BitLinear weights are sent as their
ternary {-1,0,+1} values (EXACT in fp8e4), gamma_q*gamma_k is folded into the
softmax exp scale, and 2*gamma_v*gamma_o is applied on the host (the V
weights carry an extra 0.5 — exact in bf16 — so that |v~| stays < 240, the
TRN fp8e4 saturation point).

fp8 DoubleRow (2 contraction subtiles per MM, ~1.8x MAC throughput) is used
where a numpy error simulation showed it is safe (sim rel err 0.92% vs the
2e-2 gate; all-fp8 variants fail):
  - Q/K projections:  fp8 DR (x as e4m3 moving operand, ternary w stationary)
  - V projection:     bf16, x STATIONARY per token-tile so V lands
                      token-major directly — kills the 128 PE transposes the
                      baseline spent ~35us on. w_v is the moving operand.
  - QK^T scores:      bf16 (contraction is d_head=128 — DR inapplicable)
  - P@V off-diagonal: fp8 DR (exp emits e4m3, V kept in an fp8 copy)
  - P@V diagonal:     bf16 (peaked early-token rows need accurate V)
  - o_proj:           tokens 0-127 bf16 (the absmax-setting peaked rows),
                      tokens 128+ fp8 DR
Scheduling: TWO heads' attention runs staggered, each block split into
phase A (all S matmuls + exps; P parked in SBUF) and phase B (all PV /
row-sum matmuls) so a backlogged activation engine can never stall the PE;
projection/V/o_proj fill chains are pumped between steps. Off-diagonal
row-sums ride the PE (fp8-DR all-ones stationary into a PSUM bank);
diagonal partials accumulate on DVE and one ones-matmul per block does the
cross-partition reduce+broadcast; normalization via fast reciprocal.
x8 is loaded as one tile per DR pair so the first projection matmul starts
after ~1/8 of the transfer; outputs stream back as bf16.
"""

import math

import ml_dtypes
import numpy as np

import concourse.mybir as mybir
import concourse.tile as tile
from concourse import bacc
from concourse import bass_utils

BF16 = mybir.dt.bfloat16
F32 = mybir.dt.float32
F8 = mybir.dt.float8e4
DR = mybir.MatmulPerfMode.DoubleRow
EXP = mybir.ActivationFunctionType.Exp

D_MODEL = 2048
N_HEAD = 16
D_HEAD = 128
B = 4
T_FULL = 2048
N_CORES = 8
F_LOC = D_MODEL // 2  # features per core (8 heads)


def build_bass(scale, T=T_FULL, D=D_MODEL, F=F_LOC, debug=False):
    """Build the single-core program (SPMD across 8 cores via input data)."""
    P = 128
    KD = D // P      # contraction 128-tiles (16)
    JD = KD // 2     # DR pairs over contraction (8)
    TT = T // P      # token 128-tiles
    TB = T // 512    # token 512-blocks
    H = F // P       # local heads (8)
    MT = D // P      # output-dmodel 128-tiles (16)
    KT_PER_B = 512 // P

    nc = bacc.Bacc("TRN2", target_bir_lowering=False, debug=debug,
                   num_devices=N_CORES)
    # x8/wvm arrive pre-arranged in their exact SBUF layouts (single DMA)
    x8_d = nc.dram_tensor("x8", [P, KD * T], F8, kind="ExternalInput").ap()
    xtc_d = nc.dram_tensor("xtc", [TT, P, KD * P], BF16,
                           kind="ExternalInput").ap()
    # wq8/wk8: [H, 128, KD*128] ternary fp8, [h, p, kd*128+f] = t[h*128+f, kd*128+p]
    wq8_d = nc.dram_tensor("wq8", [H, P, KD * P], F8, kind="ExternalInput").ap()
    wk8_d = nc.dram_tensor("wk8", [H, P, KD * P], F8, kind="ExternalInput").ap()
    # wvm: [128, KD*F] bf16, [p, kd*F+f] = 0.5 * tv[f, kd*128+p] (moving op)
    wvm_d = nc.dram_tensor("wvm", [P, KD * F], BF16, kind="ExternalInput").ap()
    # wo: [MT, 128, H*128], [m, p, h*128+j] = to[m*128+j, h*128+p]
    wo8_d = nc.dram_tensor("wo8", [MT, P, H * P], F8, kind="ExternalInput").ap()
    wob_d = nc.dram_tensor("wob", [MT, P, H * P], BF16,
                           kind="ExternalInput").ap()
    cm_d = nc.dram_tensor("cmask", [P, P], BF16, kind="ExternalInput").ap()
    out_d = nc.dram_tensor("outT", [D, T], BF16, kind="ExternalOutput").ap()

    with tile.TileContext(nc) as tc:
        with (
            tc.tile_pool(name="big", bufs=1) as big,
            tc.tile_pool(name="work", bufs=2) as work,
            tc.tile_pool(name="psS", bufs=2, space="PSUM") as psS,
            tc.tile_pool(name="psO", bufs=2, space="PSUM") as psO,
            tc.tile_pool(name="psR", bufs=2, space="PSUM") as psR,
            tc.tile_pool(name="psP", bufs=2, space="PSUM") as psP,
        ):
            # ---- persistent inputs (head-0 weights first: first MMs need them)
            def load_qk_weights(h):
                wq_t = work.tile([P, KD, P], F8, name=f"wq{h}", tag="wq8",
                                 bufs=3)
                nc.sync.dma_start(out=wq_t.rearrange("p kd f -> p (kd f)"),
                                  in_=wq8_d[h])
                wk_t = work.tile([P, KD, P], F8, name=f"wk{h}", tag="wk8",
                                 bufs=3)
                nc.sync.dma_start(out=wk_t.rearrange("p kd f -> p (kd f)"),
                                  in_=wk8_d[h])
                return wq_t, wk_t

            ws_list = [None] * (H + 3)
            ws_list[0] = load_qk_weights(0)
            # x8 as one tile per DR kd-pair: dependencies stay per-pair, so
            # the first projection matmul starts after ~1/8 of the transfer
            x8p = [big.tile([P, 2, T], F8, name=f"x8p{j}", tag="x8p", bufs=JD)
                   for j in range(JD)]
            for j in range(JD):
                nc.sync.dma_start(
                    out=x8p[j].rearrange("p a t -> p (a t)"),
                    in_=x8_d[:, 2 * j * T:(2 * j + 2) * T])
            if H > 1:
                ws_list[1] = load_qk_weights(1)
            if H > 2:
                ws_list[2] = load_qk_weights(2)
            wv_sb = big.tile([P, KD, F], BF16, name="wv_sb", tag="wv", bufs=1)
            nc.sync.dma_start(
                out=wv_sb.rearrange("p kd f -> p (kd f)"), in_=wvm_d)
            ones = big.tile([P, P], BF16, name="ones_sb", tag="ones", bufs=1)
            nc.vector.memset(ones, 1.0)
            ones8 = big.tile([P, 2, P], F8, name="ones8_sb", tag="ones8",
                             bufs=1)
            nc.vector.memset(ones8, 1.0)
            cmask = big.tile([P, P], BF16, name="cmask_sb", tag="cmask", bufs=1)
            nc.sync.dma_start(out=cmask, in_=cm_d)
            vhb = big.tile([P, TT, F], BF16, name="vhb", tag="vhb", bufs=1)
            vh8 = big.tile([P, TT, F], F8, name="vh8", tag="vh8", bufs=1)
            # only tokens 0-127 (the sharply peaked rows) keep a bf16 o_proj
            # path; everything else consumes the fp8 copy
            otb = big.tile([P, H, P], BF16, name="otb", tag="otb", bufs=1)
            ot8 = big.tile([P, H, T - P], F8, name="ot8", tag="ot8", bufs=1)

            chunks = {}

            def load_chunk(tt):
                c = work.tile([P, KD * P], BF16, name=f"xtc{tt}", tag="xtc")
                nc.sync.dma_start(out=c, in_=xtc_d[tt])
                chunks[tt] = c

            load_chunk(0)

            # ---- fill generators (pumped between exp-gated attention ops)
            qk_done = {}

            def qk_fill_gen(h, ws, tiles):
                wq_t, wk_t = ws
                qt_, kt_ = tiles
                for w_t, dst in ((wq_t, qt_), (wk_t, kt_)):
                    for tb in range(TB):
                        ts_ = slice(tb * 512, (tb + 1) * 512)
                        ps = psP.tile([P, 512], F32, name="psfill", tag="psp")
                        for j in range(JD):
                            nc.tensor.matmul(ps,
                                             lhsT=w_t[:, 2 * j:2 * j + 2, :],
                                             rhs=x8p[j][:, :, ts_],
                                             start=(j == 0), stop=(j == JD - 1),
                                             perf_mode=DR)
                            yield
                        nc.vector.tensor_copy(out=dst[:, ts_], in_=ps)
                qk_done[h] = True

            vdone = {}

            def v_gen(tt):
                """V projection for token-tile tt (all heads), token-major:
                x chunk stationary, w_v moving. Prefetches chunk tt+1."""
                if tt + 1 < TT:
                    load_chunk(tt + 1)
                c = chunks[tt]
                for fc in range(F // 512):
                    fs = slice(fc * 512, (fc + 1) * 512)
                    ps = psP.tile([P, 512], F32, name="psv", tag="psp")
                    for kd in range(KD):
                        nc.tensor.matmul(ps, lhsT=c[:, kd * P:(kd + 1) * P],
                                         rhs=wv_sb[:, kd, fs],
                                         start=(kd == 0), stop=(kd == KD - 1))
                        yield
                    nc.vector.tensor_copy(out=vhb[:, tt, fs], in_=ps)
                    nc.vector.tensor_copy(out=vh8[:, tt, fs], in_=ps)
                del chunks[tt]
                vdone[tt] = True

            def alloc_head_tiles(h):
                qt_ = work.tile([P, T], BF16, name=f"qt{h}", tag="qt", bufs=3)
                kt_ = work.tile([P, T], BF16, name=f"kt{h}", tag="kt", bufs=3)
                return qt_, kt_

            def oproj_nb_gen(nb):
                """o_proj chains for token block nb (nb=0 bf16, else fp8 DR).
                Weights prefetched 3 m-tiles ahead so the drain never stalls
                on a weight DMA."""
                PF = 4 if nb else 2
                ns = slice(nb * 512, (nb + 1) * 512)
                ns8 = slice(nb * 512 - P, nb * 512 - P + 512)
                wts = {}

                def load_wt(m):
                    # weight loads ride the (idle) gpsimd software-DGE queue
                    # so they never stall behind output DMAs on sync
                    wt8 = work.tile([P, H, P], F8, name=f"wo8_{nb}_{m}",
                                    tag="wo8", bufs=5)
                    nc.gpsimd.dma_start(
                        out=wt8.rearrange("p h f -> p (h f)"), in_=wo8_d[m])
                    if nb == 0:
                        wtb = work.tile([P, H, P], BF16, name=f"wob{m}",
                                        tag="wob", bufs=3)
                        nc.gpsimd.dma_start(
                            out=wtb.rearrange("p h f -> p (h f)"),
                            in_=wob_d[m])
                        wts[m] = (wt8, wtb)
                    else:
                        wts[m] = wt8

                for m in range(PF):
                    load_wt(m)
                for m in range(MT):
                    if m + PF < MT:
                        load_wt(m + PF)
                    wt = wts.pop(m)
                    yield
                    stg = work.tile([P, 512], BF16, name="ostage",
                                    tag="ostage", bufs=3)
                    if nb == 0:
                        # tokens 0-127 bf16 (otb), 128-511 fp8 DR (ot8)
                        wt8, wtb = wt
                        ps1 = psP.tile([P, P], F32, name="psout0", tag="psp")
                        for hh in range(H):
                            nc.tensor.matmul(ps1, lhsT=wtb[:, hh, :],
                                             rhs=otb[:, hh, :],
                                             start=(hh == 0),
                                             stop=(hh == H - 1))
                            yield
                        ps2 = psP.tile([P, 384], F32, name="psout1", tag="psp")
                        for j in range(H // 2):
                            nc.tensor.matmul(ps2,
                                             lhsT=wt8[:, 2 * j:2 * j + 2, :],
                                             rhs=ot8[:, 2 * j:2 * j + 2,
                                                     0:384],
                                             start=(j == 0),
                                             stop=(j == H // 2 - 1),
                                             perf_mode=DR)
                            yield
                        nc.vector.tensor_copy(out=stg[:, :P], in_=ps1)
                        nc.vector.tensor_copy(out=stg[:, P:], in_=ps2)
                    else:
                        ps = psP.tile([P, 512], F32, name="psout", tag="psp")
                        for j in range(H // 2):
                            nc.tensor.matmul(ps, lhsT=wt[:, 2 * j:2 * j + 2, :],
                                             rhs=ot8[:, 2 * j:2 * j + 2, ns8],
                                             start=(j == 0),
                                             stop=(j == H // 2 - 1),
                                             perf_mode=DR)
                            yield
                        nc.vector.tensor_copy(out=stg, in_=ps)
                    nc.sync.dma_start(out=out_d[m * P:(m + 1) * P, ns],
                                      in_=stg)

            def pump(gen, n):
                for _ in range(n):
                    try:
                        next(gen)
                    except StopIteration:
                        return False
                return True

            def pump_n(gen, n):
                c = 0
                for _ in range(n):
                    try:
                        next(gen)
                        c += 1
                    except StopIteration:
                        break
                return c

            fills = []

            def pump_fills(n):
                while n > 0 and fills:
                    n -= pump_n(fills[0], n)
                    if n > 0:
                        fills.pop(0)

            def ensure_v(tt_hi):
                while not vdone.get(tt_hi, False) and fills:
                    pump_fills(16)

            def att_gen(h):
                """Causal attention for head h, yielding at pacing points so
                the driver can interleave TWO heads (each head's S/PV covers
                the other's exp latency) plus fill chains."""
                qt_, kt_ = head_tiles[h]
                hs = slice(h * P, (h + 1) * P)
                for qb in range(TB):
                    if h <= 1:
                        ensure_v(KT_PER_B * (qb + 1) - 1)
                    nkt = KT_PER_B * (qb + 1)
                    off = KT_PER_B * qb  # off-diagonal key tiles (fp8 DR)
                    qs_full = slice(qb * 512, (qb + 1) * 512)
                    # ---- phase A: all S matmuls + exps of this block; P
                    # lands in SBUF (fp8 pairs off-diagonal, bf16 diagonal).
                    # PE work here is light (S only) — fills pumped between
                    # S ops keep it fed while ACT churns through the exps.
                    pt8s = []
                    for j in range(off // 2):
                        pt8_t = work.tile([P, 2, 512], F8, name="pt8",
                                          tag="pt8", bufs=6)
                        pt8s.append(pt8_t)
                        for u in range(2):
                            kt = 2 * j + u
                            psS_t = psS.tile([P, 512], F32, name="pssc",
                                             tag="pss")
                            nc.tensor.matmul(psS_t,
                                             lhsT=kt_[:, kt * P:(kt + 1) * P],
                                             rhs=qt_[:, qs_full],
                                             start=True, stop=True)
                            nc.scalar.activation(out=pt8_t[:, u, :],
                                                 in_=psS_t, func=EXP,
                                                 scale=scale)
                            yield
                    ptd = work.tile([P, KT_PER_B, 512], BF16, name="ptd",
                                    tag="ptd", bufs=2)
                    for di in range(KT_PER_B):
                        kt = off + di
                        c0 = di * P
                        w = 512 - c0
                        qs = slice(qb * 512 + c0, (qb + 1) * 512)
                        psS_t = psS.tile([P, 512], F32, name="pssc", tag="pss")
                        nc.tensor.matmul(psS_t[:, :w],
                                         lhsT=kt_[:, kt * P:(kt + 1) * P],
                                         rhs=qt_[:, qs],
                                         start=True, stop=True)
                        nc.scalar.activation(out=ptd[:, di, :w],
                                             in_=psS_t[:, :w],
                                             func=EXP, scale=scale)
                        nc.vector.tensor_mul(ptd[:, di, :P], ptd[:, di, :P],
                                             cmask)
                        yield
                    # ---- phase B: PV + row-sum matmuls (dense PE, no exp
                    # dependency nearer than a full phase — ACT backlog can't
                    # stall it; the paired head's phase A overlaps here)
                    psO_t = psO.tile([P, 512], F32, name="psodt", tag="pso")
                    psR_t = (psR.tile([P, 512], F32, name="psrow", tag="psr")
                             if off > 0 else None)
                    for j in range(off // 2):
                        pt8_t = pt8s[j]
                        nc.tensor.matmul(psO_t,
                                         lhsT=vh8[:, 2 * j:2 * j + 2, hs],
                                         rhs=pt8_t,
                                         start=(j == 0), stop=False,
                                         perf_mode=DR, skip_group_check=True)
                        nc.tensor.matmul(psR_t, lhsT=ones8, rhs=pt8_t,
                                         start=(j == 0), stop=False,
                                         perf_mode=DR, skip_group_check=True)
                        yield
                    racc = work.tile([P, 512], F32, name="racc", tag="racc",
                                     bufs=2)
                    for di in range(KT_PER_B):
                        kt = off + di
                        c0 = di * P
                        w = 512 - c0
                        nc.tensor.matmul(psO_t[:, c0:], lhsT=vhb[:, kt, hs],
                                         rhs=ptd[:, di, :w],
                                         start=(kt == 0),
                                         stop=(kt == nkt - 1),
                                         skip_group_check=True)
                        if di == 0:
                            nc.vector.tensor_copy(out=racc, in_=ptd[:, 0, :])
                        else:
                            nc.vector.tensor_add(racc[:, c0:], racc[:, c0:],
                                                 ptd[:, di, :w])
                        if di & 1:
                            yield
                    # cross-partition reduce the diagonal partials into psR
                    # (one ones-matmul per block; closes the psR group)
                    raccb = work.tile([P, 512], BF16, name="raccb",
                                      tag="raccb")
                    nc.vector.tensor_copy(out=raccb, in_=racc)
                    if psR_t is None:
                        psR_t = psR.tile([P, 512], F32, name="psrow",
                                         tag="psr")
                    nc.tensor.matmul(psR_t, lhsT=ones, rhs=raccb,
                                     start=(off == 0), stop=True,
                                     skip_group_check=True)
                    nc.vector.reciprocal_approx_fast(out=racc, in_=psR_t)
                    if qb == 0:
                        nc.vector.tensor_mul(otb[:, h, :], psO_t[:, :P],
                                             racc[:, :P])
                        nc.vector.tensor_mul(ot8[:, h, 0:384], psO_t[:, P:],
                                             racc[:, P:])
                    else:
                        nc.vector.tensor_mul(
                            ot8[:, h, qb * 512 - P:qb * 512 - P + 512],
                            psO_t, racc)
                    if h == H - 1:
                        fills.append(oproj_nb_gen(qb))
                    yield

            # head-0 Q/K projections up front, kd-pair-OUTER across all 8
            # PSUM banks: the first matmul needs only the first x8 slice, so
            # the PE starts ~2us after the x8 DMAs begin instead of waiting
            # for the whole 4MB transfer
            head_tiles = [None] * H
            head_tiles[0] = alloc_head_tiles(0)
            qt0, kt0 = head_tiles[0]
            wq0, wk0 = ws_list[0]
            qpools = [(psP, "psp"), (psP, "psp"), (psS, "pss"),
                      (psS, "pss")][:TB]
            kpools = [(psO, "pso"), (psO, "pso"), (psR, "psr"),
                      (psR, "psr")][:TB]
            psq = [pool.tile([P, 512], F32, name=f"psq{t}", tag=tag)
                   for t, (pool, tag) in enumerate(qpools)]
            psk = [pool.tile([P, 512], F32, name=f"psk{t}", tag=tag)
                   for t, (pool, tag) in enumerate(kpools)]
            for j in range(JD):
                for tb in range(TB):
                    ts_ = slice(tb * 512, (tb + 1) * 512)
                    nc.tensor.matmul(psq[tb], lhsT=wq0[:, 2 * j:2 * j + 2, :],
                                     rhs=x8p[j][:, :, ts_],
                                     start=(j == 0), stop=(j == JD - 1),
                                     perf_mode=DR, skip_group_check=True)
                    nc.tensor.matmul(psk[tb], lhsT=wk0[:, 2 * j:2 * j + 2, :],
                                     rhs=x8p[j][:, :, ts_],
                                     start=(j == 0), stop=(j == JD - 1),
                                     perf_mode=DR, skip_group_check=True)
            for tb in range(TB):
                ts_ = slice(tb * 512, (tb + 1) * 512)
                nc.vector.tensor_copy(out=qt0[:, ts_], in_=psq[tb])
                nc.vector.tensor_copy(out=kt0[:, ts_], in_=psk[tb])
            qk_done[0] = True

            # fills: qk(1) first (head 1 activates early under the staggered
            # pairing), then the V chains, then later heads' qk as they queue
            head_tiles[1] = alloc_head_tiles(1)
            fills.append(qk_fill_gen(1, ws_list[1], head_tiles[1]))
            qk_queued = 2
            for tt in range(TT):
                fills.append(v_gen(tt))

            def queue_qk_through(h_hi):
                nonlocal qk_queued
                while qk_queued < min(h_hi + 1, H):
                    hq = qk_queued
                    if hq + 1 < H and ws_list[hq + 1] is None:
                        ws_list[hq + 1] = load_qk_weights(hq + 1)
                    head_tiles[hq] = alloc_head_tiles(hq)
                    fills.append(qk_fill_gen(hq, ws_list[hq], head_tiles[hq]))
                    qk_queued += 1

            active = []
            next_h = 0
            while active or next_h < H:
                if len(active) < 2 and next_h < H:
                    h = next_h
                    queue_qk_through(h + 2)
                    if h > 0:
                        while not qk_done.get(h, False) and fills:
                            pump_fills(16)
                    active.append(att_gen(h))
                    next_h += 1
                for g in list(active):
                    try:
                        next(g)
                    except StopIteration:
                        active.remove(g)
                # solo head (start/end of the stagger) has half the chain
                # work per round — pump fills harder so o_proj keeps the PE
                # fed through the final head's exp-heavy stretch
                pump_fills(3 if len(active) >= 2 else 7)
            while fills:
                pump_fills(64)

    nc.compile()
    return nc


def _ternary(w):
    """BitLinear ternary weights + gamma: clip(round(w/gamma),-1,1), gamma."""
    w = np.asarray(w, dtype=np.float32)
    gamma = max(np.float32(np.abs(w).mean(dtype=np.float32)), np.float32(1e-5))
    q = np.clip(np.round(w / gamma), -1.0, 1.0).astype(np.float32)
    return q, gamma


def _causal_mask():
    k = np.arange(128)[:, None]
    q = np.arange(128)[None, :]
    return (k <= q).astype(np.float32).astype(ml_dtypes.bfloat16)


def _tile_qkv(t_shard):
    """[F, D] -> [H, 128, KD*128]: [h, p, kd*128+f] = t[h*128+f, kd*128+p]."""
    Fs, Ds = t_shard.shape
    a = t_shard.reshape(Fs // 128, 128, Ds // 128, 128)  # [h, f, kd, p]
    a = a.transpose(0, 3, 2, 1).reshape(Fs // 128, 128, Ds)
    return np.ascontiguousarray(a)


def _tile_wo(t_shard):
    """[D, F] -> [MT, 128, H*128]: [m, p, h*128+j] = t[m*128+j, h*128+p]."""
    Ds, Fs = t_shard.shape
    a = t_shard.reshape(Ds // 128, 128, Fs // 128, 128)  # [m, j, h, p]
    a = a.transpose(0, 3, 2, 1).reshape(Ds // 128, 128, Fs)
    return np.ascontiguousarray(a)


def _tile_xtc(xb):
    """[T, D] -> [TT, 128, KD*128]: [tt, p, kd*128+j] = x[tt*128+j, kd*128+p]."""
    T, D = xb.shape
    a = xb.reshape(T // 128, 128, D // 128, 128)  # [tt, j, kd, p]
    a = a.transpose(0, 3, 2, 1).reshape(T // 128, 128, D)
    return np.ascontiguousarray(a)


def _prep_inputs(x, wq, wk, wv, wo):
    bf = ml_dtypes.bfloat16
    f8 = ml_dtypes.float8_e4m3
    x = np.asarray(x, dtype=np.float32)
    tq, gq = _ternary(wq)
    tk, gk = _ternary(wk)
    tv, gv = _ternary(wv)
    to, go = _ternary(wo)
    scale = float(gq) * float(gk) / math.sqrt(D_HEAD)
    oscale = 2.0 * float(gv) * float(go)
    cmask = _causal_mask()

    def _part_major(a):  # [D, N] -> [128, (D//128)*N], [p, kd*N+n] = a[kd*128+p, n]
        D_, N_ = a.shape
        return np.ascontiguousarray(
            a.reshape(D_ // 128, 128, N_).transpose(1, 0, 2).reshape(128, -1))

    x8s = [_part_major(np.ascontiguousarray(x[b].T)).astype(f8)
           for b in range(B)]
    xtcs = [_tile_xtc(x[b]).astype(bf) for b in range(B)]
    shards = []
    for hg in range(2):
        rows = slice(hg * F_LOC, (hg + 1) * F_LOC)
        wvm = _part_major((0.5 * tv[rows, :]).T)
        shards.append({
            "wq8": _tile_qkv(tq[rows, :]).astype(f8),
            "wk8": _tile_qkv(tk[rows, :]).astype(f8),
            "wvm": wvm.astype(bf),
            "wo8": _tile_wo(to[:, rows]).astype(f8),
            "wob": _tile_wo(to[:, rows]).astype(bf),
        })
    in_maps = []
    for c in range(N_CORES):
        b, hg = c // 2, c % 2
        m = {"x8": x8s[b], "xtc": xtcs[b], "cmask": cmask}
        m.update(shards[hg])
        in_maps.append(m)
    return in_maps, scale, oscale


_NC_CACHE = {}


def _get_nc(scale):
    key = round(float(scale), 12)
    if key not in _NC_CACHE:
        _NC_CACHE[key] = build_bass(scale)
    return _NC_CACHE[key]


def run(x, wq, wk, wv, wo, trace=False):
    in_maps, scale, oscale = _prep_inputs(x, wq, wk, wv, wo)
    nc = _get_nc(scale)
    res = bass_utils.run_bass_kernel_spmd(
        nc, in_maps, core_ids=list(range(N_CORES)), trace=trace)
    out = np.empty((B, T_FULL, D_MODEL), dtype=np.float32)
    for b in range(B):
        a = np.asarray(res.results[2 * b]["outT"], dtype=np.float32)
        c = np.asarray(res.results[2 * b + 1]["outT"], dtype=np.float32)
        out[b] = (a + c).T * oscale
    return out, res


def kernel(x, wq, wk, wv, wo):
    out, _ = run(x, wq, wk, wv, wo)
    return out
